# revision 10
# baseline (speedup 1.0000x reference)
"""Trainium2 Bass kernel for nn_Attention_730144440595 (NormAttention block).

8 NeuronCores, data-parallel over batch (16 -> 2/core). Per core:
  - channel-LN folded: x centered on-device; per-pixel rstd folded into the
    V-side (vaug) as a per-partition multiply; g folded into weights on host.
  - QKV GEMM: q/k as 4 unpadded 128-row o-tiles (2 heads per tile); V computed
    directly TRANSPOSED via N=128 matmuls (lhsT = centered x chunk), killing
    the DMA-xbar transposes.
  - q L2-norm applied via PE-broadcast multiply; k-norm and the x8 scale fold
    into the exp's per-partition scale (no bias needed).
  - attention transposed (sim^T[j,i]); sim matmuls are K=64 row-group PAIRS
    (jc-even at one 64-row group, jc-odd at the other via DMA'd shadow tiles)
    so two sims run concurrently in the PE array.
  - attn@V contracts over partitions with a ones-column appended to V so the
    softmax denominator drops out of the same matmul; epilogue divides and
    writes heads into PAIR-PACKED [128, N] tiles so the out-proj contracts
    2 heads per K=128 matmul.  bf16 matmuls, f32 PSUM.
"""

import sys
import types

import numpy as np

B = 2
C = 256
N = 1024
HEADS = 4
D = 64
P = 128
NCORES = 8
LN_EPS = 1e-5
LOG8 = float(np.log(8.0))


def _host_consts():
    cst = np.zeros((P, 16), np.float32)
    # E_ind[cc][p, h] = 1 iff h == 2*cc + p//64   (cols 0-3 / 4-7)
    for cc in range(2):
        for p in range(P):
            cst[p, 4 * cc + 2 * cc + p // 64] = 1.0
    cst[:, 8] = 1.0 / 256.0    # rhs_x col0 (mean)
    cst[:, 11] = 1.0 / 256.0   # rhs_q col1 (msq)  (cols 10-11)
    cst[:, 12] = -1.0 / 256.0  # negones
    cst[:64, 13] = 1.0         # khalf indicator (rows 0-63)
    cst[64:, 15] = 1.0         # khalf_odd (rows 64-127)
    cst4 = np.zeros((HEADS, 2 * P), np.float32)
    for cc in range(2):
        for m in range(P):
            cst4[2 * cc + m // 64, cc * P + m] = 1.0
    return cst, cst4


def _host_weights(w_qkv, w_out, g):
    """wqk [C, 512] = (w_qkv^T * g)[:, :512]; wv [C, 256]; wot [256, C] = w_out^T."""
    import ml_dtypes
    wt = np.ascontiguousarray(np.asarray(w_qkv, np.float32).T)  # [C, 768]
    wt = wt * np.asarray(g, np.float32).reshape(C, 1)
    wqk = np.ascontiguousarray(wt[:, 0:512])
    wv = np.ascontiguousarray(wt[:, 512:768])
    wot = np.ascontiguousarray(np.asarray(w_out, np.float32).T)  # [ci, co]
    return (wqk.astype(ml_dtypes.bfloat16), wv.astype(ml_dtypes.bfloat16),
            wot.astype(ml_dtypes.bfloat16))


def _install_ntff_hook():
    try:
        import antenv
        if getattr(antenv, "axon_hooks", None) is not None:
            return
        from trn_agent_boot.trn_boot import _ntff_profile_via_ctypes
        hook = _ntff_profile_via_ctypes('/opt/axon/libaxon_pjrt.so')
        mod = types.ModuleType('antenv.axon_hooks')
        mod._hook = hook
        mod.get_axon_ntff_profile_hook = lambda: mod._hook
        mod.set_axon_ntff_profile_hook = lambda h: setattr(mod, '_hook', h)
        sys.modules['antenv.axon_hooks'] = mod
        antenv.axon_hooks = mod
    except Exception:
        pass


def build_nc():
    import concourse.bass as bass
    import concourse.tile as tile
    import concourse.mybir as mybir
    from concourse import bacc
    from contextlib import ExitStack

    dt = mybir.dt
    f32 = dt.float32
    bf16 = dt.bfloat16
    AF = mybir.ActivationFunctionType
    OP = mybir.AluOpType

    # Keep Exp/Ln only in the combined set so the ACT table never thrashes.
    from concourse.hw_specs import get_activation_tables
    _tabs = get_activation_tables("gen3")
    for _name, _fns in _tabs.items():
        if _name != "natural_log_exp_and_others":
            _fns.discard(AF.Exp)
            _fns.discard(AF.Ln)

    nc = bacc.Bacc("TRN2", target_bir_lowering=False, num_devices=NCORES)
    x_d = nc.dram_tensor("x", [B, C, N], f32, kind="ExternalInput").ap()
    wqk_d = nc.dram_tensor("wqk", [C, 512], bf16, kind="ExternalInput").ap()
    wv_d = nc.dram_tensor("wv", [C, 256], bf16, kind="ExternalInput").ap()
    wot_d = nc.dram_tensor("wot", [2 * P, C], bf16, kind="ExternalInput").ap()
    xbf_d = nc.dram_tensor("xbf", [B, C, N], bf16, kind="ExternalInput").ap()
    cst_d = nc.dram_tensor("cst", [P, 16], f32, kind="ExternalInput").ap()
    cst4_d = nc.dram_tensor("cst4", [HEADS, 2 * P], f32, kind="ExternalInput").ap()
    out_d = nc.dram_tensor("out", [B, C, N], f32, kind="ExternalOutput").ap()

    with tile.TileContext(nc) as tc, ExitStack() as ctx:
        const = ctx.enter_context(tc.tile_pool(name="const", bufs=1))
        big = ctx.enter_context(tc.tile_pool(name="big", bufs=1))
        tmp = ctx.enter_context(tc.tile_pool(name="tmp", bufs=2))
        expp = ctx.enter_context(tc.tile_pool(name="expp", bufs=6))
        psA = ctx.enter_context(tc.tile_pool(name="psA", bufs=3, space="PSUM"))
        psB = ctx.enter_context(tc.tile_pool(name="psB", bufs=2, space="PSUM"))

        def mm(out, lhsT, rhs, start, stop):
            nc.tensor.matmul(out, lhsT, rhs, start=start, stop=stop)

        # ---------------- constants ----------------
        cst_f = tmp.tile([P, 16], f32, tag="cst_f", name="cst_f")
        nc.sync.dma_start(cst_f, cst_d[:])
        cst = const.tile([P, 16], bf16, tag="cst", name="cst")
        nc.vector.tensor_copy(out=cst[:], in_=cst_f[:])
        E_ind = [cst[:, 0:4], cst[:, 4:8]]
        rhs_x = cst[:, 8:10]
        rhs_q = cst[:, 10:12]
        negones = cst[:, 12:13]
        khalf = [cst[:, 13:14], cst[:, 15:16]]
        cst4_f = tmp.tile([HEADS, 2 * P], f32, tag="cst4_f", name="cst4_f")
        nc.sync.dma_start(cst4_f, cst4_d[:])
        cst4 = const.tile([HEADS, 2 * P], bf16, tag="cst4", name="cst4")
        nc.vector.tensor_copy(out=cst4[:], in_=cst4_f[:])
        E4 = [cst4[:, 0:128], cst4[:, 128:256]]

        ones2 = const.tile([P, D], bf16, tag="ones2", name="ones2")
        nc.vector.memset(ones2[:], 1.0)
        ones_row = const.tile([1, P], bf16, tag="ones_row", name="ones_row")
        nc.vector.memset(ones_row[:], 1.0)
        eps_col = const.tile([P, 1], f32, tag="eps_col", name="eps_col")
        nc.vector.memset(eps_col[:], LN_EPS)
        log8_col = const.tile([P, 1], f32, tag="log8_col", name="log8_col")
        nc.vector.memset(log8_col[:], LOG8)

        # ---------------- loads ----------------
        x_sb = [[big.tile([P, N], f32, tag=f"x{b}{cc}", name=f"x{b}{cc}")
                 for cc in range(2)] for b in range(B)]

        wqk_sb = big.tile([P, 2, 512], bf16, tag="wqk", name="wqk")
        nc.sync.dma_start(wqk_sb, wqk_d.rearrange("(cc p) o -> p cc o", p=P))
        wv_sb = big.tile([P, 2, 256], bf16, tag="wv", name="wv")
        nc.sync.dma_start(wv_sb, wv_d.rearrange("(cc p) o -> p cc o", p=P))
        wot_sb = big.tile([P, 2, C], bf16, tag="wot", name="wot")
        nc.sync.dma_start(wot_sb, wot_d.rearrange("(pp p) o -> p pp o", p=P))

        x_bf = [[big.tile([P, N], bf16, tag=f"xbf{b}{cc}", name=f"xbf{b}{cc}")
                 for cc in range(2)] for b in range(B)]
        xsq = [[big.tile([P, N], bf16, tag=f"xsq{b}{cc}", name=f"xsq{b}{cc}")
                for cc in range(2)] for b in range(B)]
        for b in range(B):
            for cc in range(2):
                nc.sync.dma_start(x_bf[b][cc], xbf_d[b, cc * P:(cc + 1) * P, :])
                nc.vector.tensor_mul(xsq[b][cc][:], x_bf[b][cc][:], x_bf[b][cc][:])

        # ---------------- LN stats + centering (per batch) ----------------
        rstd_sb = [big.tile([P, 8], f32, tag=f"rstd{b}", name=f"rstd{b}")
                   for b in range(B)]
        negmean_row = [big.tile([1, N], bf16, tag=f"nmr{b}", name=f"nmr{b}")
                       for b in range(B)]

        def _stats_phase(b):
            st_ps = psA.tile([P, 8, 2], f32, tag="A", name="st_ps")
            for ic in range(8):
                sl = st_ps[:, ic]
                mm(sl, x_bf[b][0][:, ic * P:(ic + 1) * P], rhs_x, True, False)
                mm(sl, x_bf[b][1][:, ic * P:(ic + 1) * P], rhs_x, False, False)
                mm(sl, xsq[b][0][:, ic * P:(ic + 1) * P], rhs_q, False, False)
                mm(sl, xsq[b][1][:, ic * P:(ic + 1) * P], rhs_q, False, True)
            st_sb = tmp.tile([P, 8, 2], f32, tag="st_sb", name="st_sb")
            nc.vector.tensor_copy(out=st_sb[:], in_=st_ps[:])
            mean_v = st_sb[:, :, 0]
            msq_v = st_sb[:, :, 1]
            m2 = tmp.tile([P, 8], f32, tag="m2", name="m2")
            nc.vector.tensor_mul(m2[:], mean_v, mean_v)
            var = tmp.tile([P, 8], f32, tag="var", name="var")
            nc.vector.tensor_sub(var[:], msq_v, m2[:])
            lnv = tmp.tile([P, 8], f32, tag="lnv", name="lnv")
            nc.scalar.activation(lnv[:], var[:], AF.Ln, bias=eps_col[:])
            nc.scalar.activation(rstd_sb[b][:], lnv[:], AF.Exp, scale=-0.5)
            # negmean row + broadcast + center x in-place
            for ih in range(2):
                io = ih * 512
                nm_ps = psA.tile([1, 512], f32, tag="A", name="nm_ps")
                for cc in range(2):
                    mm(nm_ps[:], negones, x_bf[b][cc][:, io:io + 512],
                       start=(cc == 0), stop=(cc == 1))
                nc.vector.tensor_copy(out=negmean_row[b][:, io:io + 512], in_=nm_ps[:])
                nmbc_ps = psA.tile([P, 512], f32, tag="A", name="nmbc_ps")
                mm(nmbc_ps[:], ones_row[:],
                   negmean_row[b][:, io:io + 512], True, True)
                for cc in range(2):
                    nc.vector.tensor_add(x_bf[b][cc][:, io:io + 512],
                                         x_bf[b][cc][:, io:io + 512], nmbc_ps[:])

        # ---------------- QKV GEMM (q/k only, 4 unpadded o-tiles) ----------
        # o-tiles: 0-1 q (heads 01 | 23), 2-3 k (heads 01 | 23)
        qkv_sb = [[big.tile([P, N], bf16, tag=f"qkv{b}{ot}", name=f"qkv{b}{ot}")
                   for ot in range(4)] for b in range(B)]

        def _qkv_phase(b, ots=(0, 1, 2, 3)):
            for ot in ots:
                qk_ps = psA.tile([P, N], f32, tag="A", name="qk_ps")
                for ih in range(2):
                    io = ih * 512
                    mm(qk_ps[:, io:io + 512], wqk_sb[:, 0, ot * P:(ot + 1) * P],
                       x_bf[b][0][:, io:io + 512], True, False)
                    mm(qk_ps[:, io:io + 512], wqk_sb[:, 1, ot * P:(ot + 1) * P],
                       x_bf[b][1][:, io:io + 512], False, True)
                nc.vector.tensor_copy(out=qkv_sb[b][ot][:], in_=qk_ps[:])

        # ---------------- V direct-transposed + augment -------------------
        # vaug[b][pp][jc] : [128 j, 2 hh, 65] = [rstd_j * vT | ones]
        vaug = [[[big.tile([P, 2, D + 1], bf16, tag=f"va{b}{pp}{jc}",
                           name=f"va{b}{pp}{jc}")
                  for jc in range(8)] for pp in range(2)] for b in range(B)]

        def _vaug_phase(b, jcs=tuple(range(8))):
            for jc in jcs:
                vt_ps = psA.tile([P, N], f32, tag="A", name="vt_ps")
                for pp in range(2):
                    for cc in range(2):
                        mm(vt_ps[:, pp * P:(pp + 1) * P],
                           x_bf[b][cc][:, jc * P:(jc + 1) * P],
                           wv_sb[:, cc, pp * P:(pp + 1) * P],
                           start=(cc == 0), stop=(cc == 1))
                for pp in range(2):
                    va = vaug[b][pp][jc]
                    nc.vector.tensor_scalar_mul(
                        va[:, :, 0:D],
                        vt_ps[:, pp * P:(pp + 1) * P].rearrange(
                            "p (hh d) -> p hh d", hh=2),
                        rstd_sb[b][:, jc:jc + 1])
                    nc.gpsimd.memset(va[:, :, D:D + 1], 1.0)

        # ---------------- q/k norms ----------------
        b8_sb = [big.tile([P, 8, HEADS], f32, tag=f"b8{b}", name=f"b8{b}")
                 for b in range(B)]
        a_sb = [tmp.tile([HEADS, N], bf16, tag="a_sb", name=f"a_sb{b}")
                for b in range(B)]

        def _norm_phase(b):
            qsq = [tmp.tile([P, N], bf16, tag=f"qsq{cc}", name=f"qsq{cc}")
                   for cc in range(2)]
            for cc in range(2):
                nc.vector.tensor_mul(qsq[cc][:], qkv_sb[b][cc][:], qkv_sb[b][cc][:])
            ksq = [tmp.tile([P, N], bf16, tag=f"ksq{pp}", name=f"ksq{pp}")
                   for pp in range(2)]
            for pp in range(2):
                nc.vector.tensor_mul(ksq[pp][:], qkv_sb[b][2 + pp][:],
                                     qkv_sb[b][2 + pp][:])
            a_ln = tmp.tile([HEADS, N], f32, tag="a_ln", name="a_ln")
            for ih in range(2):
                io = ih * 512
                s2q_ps = psA.tile([HEADS, 512], f32, tag="A", name="s2q_ps")
                for cc in range(2):
                    mm(s2q_ps[:], E_ind[cc], qsq[cc][:, io:io + 512],
                       start=(cc == 0), stop=(cc == 1))
                nc.scalar.activation(a_ln[:, io:io + 512], s2q_ps[:], AF.Ln)
            nc.scalar.activation(a_sb[b][:], a_ln[:], AF.Exp, scale=-0.5)
            bsq_ps = psA.tile([P, 8, HEADS], f32, tag="A", name="bsq_ps")
            for jc in range(8):
                for h in range(HEADS):
                    mm(bsq_ps[:, jc, h:h + 1],
                       ksq[h // 2][:, jc * P:(jc + 1) * P],
                       khalf[h % 2], True, True)
            b8ln = tmp.tile([P, 8, HEADS], f32, tag="b8ln", name="b8ln")
            nc.scalar.activation(b8ln[:], bsq_ps[:], AF.Ln)
            nc.scalar.activation(b8_sb[b][:], b8ln[:], AF.Exp, scale=-0.5,
                                 bias=log8_col[:])
            for cc in range(2):
                for ih in range(2):
                    io = ih * 512
                    abc_ps = psA.tile([P, 512], f32, tag="A", name="abc_ps")
                    mm(abc_ps[:], E4[cc], a_sb[b][:, io:io + 512], True, True)
                    nc.vector.tensor_mul(qkv_sb[b][cc][:, io:io + 512],
                                         qkv_sb[b][cc][:, io:io + 512], abc_ps[:])

        # ---------------- shadow tiles (other 64-row placement) ------------
        # For head h (natural rows 64*hh..): shadow holds q/k at rows
        # 64*(1-hh).. so jc-odd sims run in the other PE row group.
        def _shadow(b, h):
            pp, hh = h // 2, h % 2
            qs = tmp.tile([P, N], bf16, tag="qsh", name=f"qsh{b}{h}")
            ks = tmp.tile([P, N], bf16, tag="ksh", name=f"ksh{b}{h}")
            so, do = hh * D, (1 - hh) * D
            nc.sync.dma_start(qs[do:do + D, :], qkv_sb[b][pp][so:so + D, :])
            nc.sync.dma_start(ks[do:do + D, :], qkv_sb[b][2 + pp][so:so + D, :])
            return qs, ks

        # ---------------- attention ----------------
        u65 = [[big.tile([D + 1, N], bf16, tag=f"u{b}{h}", name=f"u{b}{h}")
                for h in range(HEADS)] for b in range(B)]
        ut = [[big.tile([P, N], bf16, tag=f"ut{b}{pp}", name=f"ut{b}{pp}")
               for pp in range(2)] for b in range(B)]
        for b in range(B):
            for cc in range(2):
                nc.sync.dma_start(x_sb[b][cc], x_d[b, cc * P:(cc + 1) * P, :])

        def _issue_sims(b, h, p, qs, ks):
            """Issue the 4 sim matmuls for jc pair (2p, 2p+1) as K=64
            row-group pairs (jc-even natural rows, jc-odd shadow rows) so
            two sims run concurrently in the PE array."""
            pp, hh = h // 2, h % 2
            nat, sho = hh * D, (1 - hh) * D
            kt, qt = qkv_sb[b][2 + pp], qkv_sb[b][pp]
            je, jo = 2 * p, 2 * p + 1
            tiles = [psA.tile([P, N], f32, tag="A", name=f"sim{jc}")
                     for jc in (je, jo)]
            for ih in range(2):
                io = ih * 512
                for t, jc, sk, sq, ro in ((tiles[0], je, kt, qt, nat),
                                          (tiles[1], jo, ks, qs, sho)):
                    mm(t[:, io:io + 512],
                       sk[ro:ro + D, jc * P:(jc + 1) * P],
                       sq[ro:ro + D, io:io + 512],
                       True, True)
            return tiles

        def _exps(b, h, p, sims):
            ets = []
            for ji, jc in ((0, 2 * p), (1, 2 * p + 1)):
                et = expp.tile([P, N], bf16, tag="et", name="et")
                nc.scalar.activation(et[:], sims[ji][:], AF.Exp,
                                     scale=b8_sb[b][:, jc, h:h + 1])
                ets.append(et)
            return ets

        def _attv(b, h, p, U_ps, ets):
            pp, hh = h // 2, h % 2
            for ji, jc in ((0, 2 * p), (1, 2 * p + 1)):
                for ih in range(2):
                    mm(U_ps[ih][:], vaug[b][pp][jc][:, hh],
                       ets[ji][:, ih * 512:(ih + 1) * 512],
                       start=(jc == 0), stop=(jc == 7))

        def _epi_h(b, h):
            pp, hh = h // 2, h % 2
            for ih in range(2):
                io = ih * 512
                sbc_ps = psA.tile([D, 512], f32, tag="A", name="sbc_ps")
                mm(sbc_ps[:], ones2[D:D + 1, :],
                   u65[b][h][D:D + 1, io:io + 512], True, True)
                rbc = tmp.tile([D, 512], f32, tag="rbc", name="rbc")
                nc.vector.reciprocal_approx_fast(out=rbc[:], in_=sbc_ps[:])
                nc.gpsimd.tensor_mul(ut[b][pp][hh * D:(hh + 1) * D, io:io + 512],
                                     u65[b][h][0:D, io:io + 512], rbc[:])

        def _proj(b, cos=(0, 1)):
            for co in cos:
                out_f = tmp.tile([P, N], f32, tag="out_f", name="out_f")
                for ih in range(2):
                    io = ih * 512
                    out_ps = psA.tile([P, 512], f32, tag="A", name="out_ps")
                    for pp in range(2):
                        mm(out_ps[:], wot_sb[:, pp, co * P:(co + 1) * P],
                           ut[b][pp][:, io:io + 512],
                           start=(pp == 0), stop=(pp == 1))
                    nc.vector.tensor_add(out_f[:, io:io + 512],
                                         out_ps[:], x_sb[b][co][:, io:io + 512])
                nc.sync.dma_start(out_d[b, co * P:(co + 1) * P, :], out_f[:])

        # ---------------- schedule ----------------
        # Software-pipelined attention: per unit (b, h, jc-pair) the exps for
        # the current pair issue first on ACT, then the NEXT pair's sims on
        # PE (ahead of the exp-dependent attnV matmuls, so the PE queue never
        # head-of-line-blocks the exp stream), then attnV.  b1 prep phases,
        # epilogues and projections are woven into the PE/ACT slack.
        _stats_phase(0)
        _qkv_phase(0)
        _vaug_phase(0)
        _norm_phase(0)

        shadows = {(0, 0): _shadow(0, 0)}
        side = {
            (0, 0, 0): lambda: (_stats_phase(1),),
            (0, 0, 1): lambda: shadows.update({(0, 1): _shadow(0, 1)}),
            (0, 0, 2): lambda: (_qkv_phase(1, (0, 1)),),
            (0, 1, 0): lambda: (shadows.update({(0, 2): _shadow(0, 2)}),
                                _qkv_phase(1, (2, 3))),
            (0, 1, 2): lambda: (_vaug_phase(1, (0, 1, 2, 3)),),
            (0, 2, 0): lambda: (shadows.update({(0, 3): _shadow(0, 3)}),
                                _vaug_phase(1, (4, 5, 6, 7))),
            (0, 2, 2): lambda: (_norm_phase(1),),
            (0, 3, 0): lambda: shadows.update({(1, 0): _shadow(1, 0)}),
            (0, 3, 2): lambda: (_epi_h(0, 0),),
            (1, 0, 0): lambda: (shadows.update({(1, 1): _shadow(1, 1)}),
                                _epi_h(0, 1)),
            (1, 0, 2): lambda: (_epi_h(0, 2),),
            (1, 1, 0): lambda: (shadows.update({(1, 2): _shadow(1, 2)}),
                                _epi_h(0, 3)),
            (1, 1, 2): lambda: (_proj(0, (0,)),),
            (1, 2, 0): lambda: (shadows.update({(1, 3): _shadow(1, 3)}),
                                _proj(0, (1,))),
            (1, 2, 2): lambda: (_epi_h(1, 0),),
            (1, 3, 0): lambda: (_epi_h(1, 1),),
            (1, 3, 2): lambda: (_epi_h(1, 2),),
        }
        units = [(b, h, p) for b in range(B) for h in range(HEADS)
                 for p in range(4)]
        pend = _issue_sims(*units[0], *shadows[(0, 0)])
        U_ps = None
        for i, (b, h, p) in enumerate(units):
            if p == 0:
                U_ps = [psB.tile([D + 1, 512], f32, tag="B", name=f"U{ih}")
                        for ih in range(2)]
            ets = _exps(b, h, p, pend)
            if i + 1 < len(units):
                nb, nh, np_ = units[i + 1]
                pend = _issue_sims(nb, nh, np_, *shadows[(nb, nh)])
            _attv(b, h, p, U_ps, ets)
            if p == 3:
                for ih in range(2):
                    nc.vector.tensor_copy(
                        out=u65[b][h][:, ih * 512:(ih + 1) * 512],
                        in_=U_ps[ih][:])
            hook = side.get((b, h, p))
            if hook is not None:
                hook()
        _epi_h(1, 3)
        _proj(1)

    nc.compile()
    return nc


_NC = None
last_exec_time_ns = None


def _get_nc():
    global _NC
    if _NC is None:
        _NC = build_nc()
    return _NC


def _run(in_maps, trace=False):
    global last_exec_time_ns
    from concourse.bass_utils import run_bass_kernel_spmd
    nc = _get_nc()
    if trace:
        _install_ntff_hook()
    try:
        res = run_bass_kernel_spmd(nc, in_maps, core_ids=list(range(NCORES)),
                                   trace=trace)
    except Exception:
        if not trace:
            raise
        res = run_bass_kernel_spmd(nc, in_maps, core_ids=list(range(NCORES)),
                                   trace=False)
    last_exec_time_ns = res.exec_time_ns
    return res


def kernel(x, g, w_qkv, w_out, _trace=False):
    x = np.ascontiguousarray(np.asarray(x, dtype=np.float32))
    g = np.asarray(g, dtype=np.float32).reshape(C)
    wqk, wv, wot = _host_weights(w_qkv, w_out, g)
    b_full, c, H, W = x.shape
    assert (b_full, c, H * W) == (NCORES * B, C, N)
    xr = x.reshape(b_full, C, N)
    cst, cst4 = _host_consts()
    in_maps = []
    for i in range(NCORES):
        import ml_dtypes as _md
        in_maps.append({
            "x": np.ascontiguousarray(xr[i * B:(i + 1) * B]),
            "xbf": np.ascontiguousarray(xr[i * B:(i + 1) * B]).astype(_md.bfloat16),
            "wqk": wqk,
            "wv": wv,
            "wot": wot,
            "cst": cst,
            "cst4": cst4,
        })
    res = _run(in_maps, trace=_trace)
    out = np.concatenate([res.results[i]["out"] for i in range(NCORES)], axis=0)
    return out.reshape(b_full, C, H, W).astype(np.float32)
